# revision 12
# baseline (speedup 1.0000x reference)
"""Sharded GQA attention (causal + packed-segment mask) for 8 Trainium2 NeuronCores.

Strategy
--------
* Core c handles batch b = c//4 and KV heads {2*(c%4), 2*(c%4)+1} (8 query
  heads per core); the sequence dim stays unsharded.  decoder_segment_ids
  are sorted, so attention is block-diagonal over contiguous segments; the
  device kernel does causal-only attention per segment over 128-wide
  chunks, with the two batches' run structures unioned so all 8 cores run
  one SPMD program.
* dtypes: QK matmuls run float16 (1 col/cycle on the PE at any moving
  size, half the Q/K DMA bytes); P (post-exp) and V are bf16 so the
  130-col PV matmuls also stream 1 col/cycle; output is bf16 (host
  upcasts).  Measured end-to-end rel err 6.5e-3 (gate 2e-2).
* Q is packed host-side to only-real columns (ghost q columns of partial
  tail blocks are never computed); QK, exp, normalize and the output DMA
  are trimmed accordingly.  Zero-padded K rows self-neutralise (S=0 ->
  P=1 but V rows and the ones-column are zero), so no segment/ghost
  masking is needed anywhere.
* The causal mask inside each diagonal 128x128 block is a single shared
  additive bf16 NEG tile accumulated into the diagonal chunk's S by an
  identity-stationary matmul in the SAME PSUM accumulation group as the
  QK (no cross-engine hop; exp then emits exact zeros).
* Per-chunk S lives in its own PSUM bank (CG=1, 4-buffer pool) and exp
  runs per chunk on ScalarE; softmax denominators fall out of the PV
  matmuls via a bf16 ones-column appended to V (P^T-stationary, output
  [128, 2, 512] 2-bank psum, double-buffered); the normalize is one
  reciprocal + one 4D broadcast tensor_mul per slab on DVE.
* Emission is a software-pipelined wavefront: the 6 independent (i,kv)
  streams are staggered by one t-block step, and stage1 (QK+mask+exp) of
  step t is emitted before stage2 (PV+normalize) of step t-1, so every
  in-order engine queue always holds dependency-resolved work.
* DMA-issue overhead (~1.2us per DMA) and the serial input phase dominate
  the single-shot time, so per-(i,kv) inputs (K^T, packed Q^T, V) ride in
  one uint16-packed DMA with bitcast views (split k+q|v, extra-fine for
  the first stream), mask/ident load once from the ACT queue, outputs
  leave per-slab, and a few dep-free warmup matmuls ramp the PE clock
  during the input-DMA dead zone.

Measured on the 8 axon-tunneled trn2 NeuronCores (two-point For_i-looped
timing, RPC-drift-immune): 32669 ns per invocation vs 102159 ns baseline
(3.13x); rel err 6.488e-3.
"""

import math

import numpy as np
import ml_dtypes

B, T, NQ, NKV, D = 2, 1024, 32, 8, 128
G = NQ // NKV
NCORES = 8
KV_PER_CORE = NKV // (NCORES // B)
CHUNK = 128
BF16 = ml_dtypes.bfloat16

QDT = "f16"           # "f32r" or "f16" for the QK matmul dtype
MASK_MODE = "pe"      # "pe": additive NEG mask matmul fused into the QK
                      # accumulation; "dve": 0/1 multiply post-exp
MASK_GP_FRAC = 0.72   # dve mode: fraction of mask multiplies on GPSIMD
NEG = -1.0e9
CG = 1                # chunks per PSUM slab tile (banks each)
SLAB_BUFS = 4         # psum_s pool buffers
OT_BUFS = 2           # psum_o pool buffers
DMA_SPLIT = "first"   # input DMA pieces: "all" = (k+q | v) per (i,kv),
                      # "first" = split only stream 0, False = whole
OUT_MODE = "ikv_pool"  # "slab": per-slab DMAs on SP; "ikv": one per (i,kv)
                       # on SP; "ikv_pool": one per (i,kv) on Pool/SWDGE
                       # (bypasses the shared HWDGE + SP sequencer)
DMA_QUEUES = 1        # spread input DMAs across SP/ACT HWDGE queues
WARMUP_MM = 10        # dummy matmuls at t=0 of a single-shot build (ramp
                      # the PE clock during the input-DMA dead zone)
WARMUP_LOOP = 0       # same, inside a For_i timed body (steady state keeps
                      # the PE warm across iterations)
INPUT_BUFS = 2        # per-(i,kv) input tile buffers; 2 lets iteration n+1
                      # prefetch its inputs under iteration n's compute
LAYOUT = "interleaved"  # packed-input column order: "interleaved"
                        # ([k0|q0|k1|q1|...|v], consumption order) or
                        # "flat" ([k|q|v] blocks)
NORM_DIV = False       # normalize as one TT divide (vs reciprocal+multiply)
CGMIX = False          # mixed slab tiles: chunk PAIRS share a 2-bank tile
                      # + one exp; singles keep 1-bank tiles (24 vs 36 exps)
SLAB_TILE = None       # "j": slab j gets ONE (j+1)-bank PSUM tile and ONE
                      # exp over all its chunks (18 exps, 6 slab banks,
                      # needs OT_BUFS=1); None: CG/CGMIX per-chunk tiles
STAGGERED_RESET = True  # For_i staggered semaphore reset instead of a hard
                        # all-engine barrier between iterations

_PROGRAM_CACHE = {}


# --------------------------------------------------------------------------
# host-side structure
# --------------------------------------------------------------------------

def _runs(seg_row):
    d = np.flatnonzero(np.diff(seg_row) != 0)
    starts = np.concatenate(([0], d + 1))
    ends = np.concatenate((d + 1, [len(seg_row)]))
    return [(int(s), int(e - s)) for s, e in zip(starts, ends)]


def _structure(ids):
    runs = [_runs(np.asarray(ids[b])) for b in range(B)]
    n_seg = max(len(r) for r in runs)
    L = [max((r[i][1] for r in runs if len(r) > i), default=0) for i in range(n_seg)]
    K = [math.ceil(l / CHUNK) for l in L]
    segs = [i for i in range(n_seg) if K[i] > 0]
    slabs = [(i, kv_i, j) for i in segs for kv_i in range(KV_PER_CORE)
             for j in range(K[i])]
    chunks = [(i, kv_i, c) for i in segs for kv_i in range(KV_PER_CORE)
              for c in range(K[i])]
    # real (non-ghost) q columns of slab (i, kv_i, j), from the union lengths
    nr = {(i, kv_i, j): min(CHUNK, L[i] - j * CHUNK)
          for (i, kv_i, j) in slabs}
    qbase = {}
    acc = 0
    for s in slabs:
        qbase[s] = acc
        acc += G * nr[s]
    return runs, L, K, segs, slabs, chunks, nr, qbase, acc


def _ikv_layout(K, slabs, chunks, nr, qbase):
    """Per-(i,kv) packed-input column layout (units: 2-byte elements).

    Columns are packed in consumption order — [k_0|q_0|k_1|q_1|...|v] —
    so the DMA pieces stream in exactly the order compute needs them.
    koff/qoff are offsets local to the (i,kv) region; voff starts the V
    block.
    """
    chunk_idx = {c: i for i, c in enumerate(chunks)}
    ikvs = sorted({(i, kv_i) for (i, kv_i, _) in slabs})
    lay = {}
    base = 0
    for (i, kv_i) in ikvs:
        kk = K[i]
        koff, qoff = [], []
        if LAYOUT == "interleaved":
            off = 0
            for j in range(kk):
                koff.append(off)
                off += CHUNK
                qoff.append(off)
                off += G * nr[(i, kv_i, j)]
        else:  # flat: [k_0..k_{kk-1} | q_0..q_{kk-1} | v]
            off = 0
            for j in range(kk):
                koff.append(j * CHUNK)
            off = kk * CHUNK
            for j in range(kk):
                qoff.append(off)
                off += G * nr[(i, kv_i, j)]
        vcols = kk * 130
        lay[(i, kv_i)] = dict(base=base, koff=koff, qoff=qoff, voff=off,
                              vcols=vcols, icols=off + vcols,
                              ci0=chunk_idx[(i, kv_i, 0)], kk=kk)
        base += off + vcols
    return ikvs, lay, base


def _prepare_core(core, q, k, v, runs, L, K, segs, slabs, chunks, nr, qbase,
                  qcols, qdt=QDT):
    b = core // (NCORES // B)
    kv_heads = [KV_PER_CORE * (core % (NCORES // B)) + x for x in range(KV_PER_CORE)]
    rb = runs[b]
    np_qdt = np.float32 if qdt == "f32r" else np.float16

    def seg_info(i):
        if i < len(rb):
            return rb[i]
        return (0, 0)

    qT = np.zeros((D, qcols), np_qdt)
    for s in slabs:
        i, kv_i, j = s
        a, lb = seg_info(i)
        t0 = j * CHUNK
        n_real = min(nr[s], max(lb - t0, 0))
        if n_real > 0:
            base = qbase[s]
            for g in range(G):
                h = G * kv_heads[kv_i] + g
                blk = q[b, a + t0:a + t0 + n_real, h, :]  # [n_real, D]
                qT[:, base + g * nr[s]: base + g * nr[s] + n_real] = blk.T

    kT = np.zeros((D, len(chunks) * CHUNK), np_qdt)
    vO = np.zeros((CHUNK, len(chunks) * 130), BF16)
    for ci, (i, kv_i, c) in enumerate(chunks):
        a, lb = seg_info(i)
        s0 = c * CHUNK
        n_real = min(CHUNK, lb - s0)
        if n_real > 0:
            kvh = kv_heads[kv_i]
            kT[:, ci * CHUNK: ci * CHUNK + n_real] = \
                k[b, a + s0:a + s0 + n_real, kvh, :].T.astype(np_qdt)
            vO[:n_real, ci * 130: ci * 130 + D] = \
                v[b, a + s0:a + s0 + n_real, kvh, :].astype(BF16)
            vO[:n_real, ci * 130 + D] = BF16(1.0)

    sr = np.arange(CHUNK)
    if MASK_MODE == "pe":
        keep = np.where(sr[:, None] > sr[None, :], np.float32(NEG),
                        np.float32(0.0))  # additive: NEG where t < s
    else:
        keep = (sr[:, None] <= sr[None, :]).astype(np.float32)  # 0/1 keep
    mask = np.concatenate([keep] * G, axis=1).astype(BF16)  # [s, g*128 + t]

    return {"qT": qT, "kT": kT, "vO": vO, "mask": mask,
            "ident": np.eye(CHUNK, dtype=BF16)}


def _pack_core(ci, K, slabs, chunks, nr, qbase, qdt=QDT):
    """Build the device in_map from the logical per-core arrays."""
    ikvs, lay, total = _ikv_layout(K, slabs, chunks, nr, qbase)
    if qdt == "f16":
        inb = np.zeros((CHUNK, total), np.uint16)
        for ikv in ikvs:
            l = lay[ikv]
            b0 = l["base"]
            ci0, kk = l["ci0"], l["kk"]
            for j in range(kk):
                inb[:, b0 + l["koff"][j]: b0 + l["koff"][j] + CHUNK] = \
                    ci["kT"][:, (ci0 + j) * CHUNK:(ci0 + j + 1) * CHUNK] \
                    .view(np.uint16)
                s = (ikv[0], ikv[1], j)
                qn = G * nr[s]
                inb[:, b0 + l["qoff"][j]: b0 + l["qoff"][j] + qn] = \
                    ci["qT"][:, qbase[s]: qbase[s] + qn].view(np.uint16)
            inb[:, b0 + l["voff"]: b0 + l["voff"] + l["vcols"]] = \
                ci["vO"][:, ci0 * 130:(ci0 + kk) * 130].view(np.uint16)
        mi = np.concatenate([ci["mask"], ci["ident"]], axis=1)
        return {"inb": inb, "mi": mi}
    mi = np.concatenate([ci["mask"], ci["ident"]], axis=1)
    return {"kT": ci["kT"], "qT": ci["qT"], "vO": ci["vO"], "mi": mi}


def _assemble(outs, runs, slabs, nr):
    full = np.zeros((B, T, NQ, D), np.float32)
    for core in range(NCORES):
        b = core // (NCORES // B)
        kv_heads = [KV_PER_CORE * (core % (NCORES // B)) + x
                    for x in range(KV_PER_CORE)]
        res = outs[core]  # [NSLAB, 128, 512] bf16
        rb = runs[b]
        for si, (i, kv_i, j) in enumerate(slabs):
            if i >= len(rb):
                continue
            a, lb = rb[i]
            t0 = j * CHUNK
            n_real = min(CHUNK, lb - t0)
            if n_real <= 0:
                continue
            for g in range(G):
                h = G * kv_heads[kv_i] + g
                full[b, a + t0:a + t0 + n_real, h, :] = \
                    res[si, :n_real, g * CHUNK:g * CHUNK + D].astype(np.float32)
    return full


# --------------------------------------------------------------------------
# numpy emulation of the device schedule (debug/validation only)
# --------------------------------------------------------------------------

def _numpy_schedule(ins, L, K, segs, slabs, chunks, nr, qbase):
    chunk_idx = {c: i for i, c in enumerate(chunks)}
    qT = ins["qT"].astype(np.float32)
    kT = ins["kT"].astype(np.float32)
    vO = ins["vO"].astype(np.float32)
    mask = ins["mask"].astype(np.float32)
    out = np.zeros((len(slabs), CHUNK, G * CHUNK), BF16)
    for si, (i, kv_i, j) in enumerate(slabs):
        n = nr[(i, kv_i, j)]
        qt = qT[:, qbase[(i, kv_i, j)]: qbase[(i, kv_i, j)] + G * n]  # [d, 4n]
        ot = np.zeros((CHUNK, G, 130), np.float32)
        for c in range(j + 1):
            ci = chunk_idx[(i, kv_i, c)]
            lhsT = kT[:, ci * CHUNK:(ci + 1) * CHUNK]          # [d, s]
            S = lhsT.T @ qt                                    # [s, 4n]
            m = np.concatenate([mask[:, :n]] * G, axis=1)      # [s, 4n]
            if MASK_MODE == "pe":
                if c == j:
                    S = S + m
                P = np.exp(S)
            else:
                P = np.exp(S)
                if c == j:
                    P = P * m
            P = P.astype(BF16).astype(np.float32)
            vo = vO[:, ci * 130:ci * 130 + 130]                # [s, 130]
            for g in range(G):
                ot[:n, g, :] += P[:, g * n:(g + 1) * n].T @ vo
        den = ot[:, :, D]
        with np.errstate(divide="ignore", invalid="ignore"):
            recip = 1.0 / den
            norm = ot[:, :, :D] * recip[:, :, None]
        out[si, :, :] = norm.reshape(CHUNK, G * D).astype(BF16)
    return out


# --------------------------------------------------------------------------
# bass program
# --------------------------------------------------------------------------

def _build_program(L, K, segs, slabs, chunks, nr, qbase, qcols, qdt=QDT,
                   loop_n=0, unroll=1):
    import contextlib

    import concourse.bacc as bacc
    import concourse.bass as bass
    import concourse.tile as tile
    from concourse import mybir

    f32 = mybir.dt.float32
    bf16 = mybir.dt.bfloat16
    u16 = mybir.dt.uint16
    f16pack = qdt == "f16"
    mm_dt = mybir.dt.float32r if qdt == "f32r" else mybir.dt.float16
    maxK = max(K[i] for i in segs)
    nslab = len(slabs)
    nchunk = len(chunks)
    ikvs, lay, packed_cols = _ikv_layout(K, slabs, chunks, nr, qbase)

    nc = bacc.Bacc()
    if f16pack:
        inb_d = nc.dram_tensor("inb", [CHUNK, packed_cols], u16,
                               kind="ExternalInput")
    else:
        qT_d = nc.dram_tensor("qT", [D, qcols], mm_dt, kind="ExternalInput")
        kT_d = nc.dram_tensor("kT", [D, nchunk * CHUNK], mm_dt,
                              kind="ExternalInput")
        vO_d = nc.dram_tensor("vO", [CHUNK, nchunk * 130], bf16,
                              kind="ExternalInput")
    mi_d = nc.dram_tensor("mi", [CHUNK, G * CHUNK + CHUNK], bf16,
                          kind="ExternalInput")
    out_d = nc.dram_tensor("out", [nslab, CHUNK, G * CHUNK], bf16,
                           kind="ExternalOutput")
    slab_idx = {s: i for i, s in enumerate(slabs)}

    with tile.TileContext(nc) as tc:
      with tc.tile_pool(name="pin", bufs=1) as pin, \
           tc.tile_pool(name="pp", bufs=3) as pp, \
           tc.tile_pool(name="po", bufs=2) as po, \
           tc.tile_pool(name="psum_s", bufs=SLAB_BUFS, space="PSUM") as psum_s, \
           tc.tile_pool(name="psum_o", bufs=OT_BUFS, space="PSUM") as psum_o:
        # loop-invariant: causal mask + identity in ONE tile/DMA; issued from
        # the ACT queue so the SP queue's first input DMA is not delayed
        mi_t = pin.tile([CHUNK, G * CHUNK + CHUNK], bf16, tag="mi")
        nc.scalar.dma_start(out=mi_t[:], in_=mi_d[:])
        mask_t = mi_t[:, 0:G * CHUNK]
        ident_t = mi_t[:, G * CHUNK:G * CHUNK + CHUNK]
        warm_t = pin.tile([CHUNK, CHUNK], bf16, tag="warm")
        nc.vector.memset(warm_t[:], 0.0)
        warm_n = WARMUP_LOOP if loop_n else WARMUP_MM
        with (tc.For_i(0, loop_n, 1, staggered_reset=STAGGERED_RESET)
              if loop_n else contextlib.nullcontext()):
          for _it in range(max(1, unroll)):
            if warm_n:
                # dep-free dummy matmuls (uninitialized operands, result
                # overwritten): keep the PE busy during the input-DMA head
                # so the HAM/pstate clock is warm for the first real QK
                wslab = psum_s.tile(
                    [CHUNK, 1 if CGMIX else CG, G * CHUNK], f32, tag="slab",
                    bufs=2 if CGMIX else SLAB_BUFS, name="wslab")
                for w in range(warm_n):
                    nc.tensor.matmul(wslab[:, 0, 0:CHUNK], warm_t[:],
                                     warm_t[:], start=True, stop=True)
            # one packed input DMA per (i,kv), in consumption order, so the
            # For_i loop's n+1 DMAs overlap iteration n's compute
            kT_t, qT_t, vO_t = {}, {}, {}
            for gi_, ikv in enumerate(ikvs):
                dma_eng = (nc.gpsimd if (DMA_QUEUES > 1 and gi_ % 2 == 1)
                           else nc.sync)
                l = lay[ikv]
                kk = l["kk"]
                if f16pack:
                    icols = l["icols"]
                    voff = l["voff"]
                    it = pin.tile([CHUNK, icols], u16,
                                  tag=f"in_{ikv[0]}_{ikv[1]}",
                                  bufs=INPUT_BUFS)
                    if DMA_SPLIT == "first" and ikv == ikvs[0]:
                        # first stream lands [k0|q0] first so the first
                        # QK starts ASAP; rest follows in one piece
                        p1 = l["qoff"][0] + G * nr[(ikv[0], ikv[1], 0)]
                        dma_eng.dma_start(
                            out=it[:, 0:p1],
                            in_=inb_d[:, l["base"]: l["base"] + p1])
                        dma_eng.dma_start(
                            out=it[:, p1:icols],
                            in_=inb_d[:, l["base"] + p1: l["base"] + icols])
                    elif DMA_SPLIT == "all":
                        dma_eng.dma_start(
                            out=it[:, 0:voff],
                            in_=inb_d[:, l["base"]: l["base"] + voff])
                        dma_eng.dma_start(
                            out=it[:, voff:icols],
                            in_=inb_d[:, l["base"] + voff: l["base"] + icols])
                    else:
                        dma_eng.dma_start(
                            out=it[:],
                            in_=inb_d[:, l["base"]: l["base"] + icols])
                    kT_t[ikv] = [it[:, l["koff"][j]: l["koff"][j] + CHUNK]
                                 .bitcast(mm_dt) for j in range(kk)]
                    qT_t[ikv] = [
                        it[:, l["qoff"][j]:
                           l["qoff"][j] + G * nr[(ikv[0], ikv[1], j)]]
                        .bitcast(mm_dt) for j in range(kk)]
                    vO_t[ikv] = it[:, voff: icols].bitcast(bf16)
                else:
                    ci0 = l["ci0"]
                    s0 = (ikv[0], ikv[1], 0)
                    qlen = sum(G * nr[(ikv[0], ikv[1], j)] for j in range(kk))
                    kt = pin.tile([D, kk * CHUNK], mm_dt,
                                  tag=f"kT_{ikv[0]}_{ikv[1]}")
                    nc.sync.dma_start(
                        out=kt[:], in_=kT_d[:, ci0 * CHUNK:(ci0 + kk) * CHUNK])
                    kT_t[ikv] = [kt[:, j * CHUNK:(j + 1) * CHUNK]
                                 for j in range(kk)]
                    qt = pin.tile([D, qlen], mm_dt,
                                  tag=f"qT_{ikv[0]}_{ikv[1]}")
                    nc.sync.dma_start(
                        out=qt[:], in_=qT_d[:, qbase[s0]: qbase[s0] + qlen])
                    qT_t[ikv] = [
                        qt[:, qbase[(ikv[0], ikv[1], j)] - qbase[s0]:
                           qbase[(ikv[0], ikv[1], j)] - qbase[s0]
                           + G * nr[(ikv[0], ikv[1], j)]]
                        for j in range(kk)]
                    vt = pin.tile([CHUNK, kk * 130], bf16,
                                  tag=f"vO_{ikv[0]}_{ikv[1]}")
                    nc.sync.dma_start(
                        out=vt[:], in_=vO_d[:, ci0 * 130:(ci0 + kk) * 130])
                    vO_t[ikv] = vt[:]

            # ---- software-pipelined wavefront over the (i,kv) streams ----
            # Streams are independent; stagger them by one j-step and emit
            # stage1 (QK+exp+mask) of step t before stage2 (PV+normalize)
            # of step t-1, so every engine's in-order queue always holds
            # dependency-resolved work.
            mask_state = {"idx": 0}
            ost_t = {}
            done_t = {}

            def stage1(i, kv_i, j):
                kt = kT_t[(i, kv_i)]
                n = nr[(i, kv_i, j)]
                fcols = G * n
                qt = qT_t[(i, kv_i)][j]
                m_ap = bass.AP(tensor=mask_t.tensor, offset=mask_t.offset,
                               ap=[mask_t.ap[0], [0, G], [1, n]])
                pts = []  # (pt_tile, c0, glen)
                step = (j + 1) if SLAB_TILE == "j" else (2 if CGMIX else CG)
                for c0 in range(0, j + 1, step):
                    glen = min(step, j + 1 - c0)
                    if SLAB_TILE == "j":
                        slab = psum_s.tile([CHUNK, j + 1, G * CHUNK], f32,
                                           tag=f"slabJ{j}", bufs=1,
                                           name=f"slabJ{j}")
                    elif CGMIX and glen == 2:
                        # paired chunks share a 2-bank tile and ONE exp
                        slab = psum_s.tile([CHUNK, 2, G * CHUNK], f32,
                                           tag="slab2", bufs=1, name="slab2")
                    elif CGMIX:
                        slab = psum_s.tile([CHUNK, 1, G * CHUNK], f32,
                                           tag="slab", bufs=2, name="slab")
                    else:
                        slab = psum_s.tile([CHUNK, CG, G * CHUNK], f32,
                                           tag="slab", name="slab")
                    for gi in range(glen):
                        c = c0 + gi
                        masked = MASK_MODE == "pe" and c == j
                        nc.tensor.matmul(
                            slab[:, gi, 0:fcols],
                            kt[c], qt,
                            start=True, stop=not masked)
                        if masked:
                            # accumulate the additive NEG causal mask into
                            # the diagonal chunk's S (same PSUM group, no
                            # cross-engine hop; exp then emits exact zeros)
                            sl3 = slab[:, gi, 0:fcols] \
                                .rearrange("p (g t) -> p g t", g=G)
                            nc.tensor.matmul(sl3, ident_t[:], m_ap,
                                             start=False, stop=True)
                    pt = pp.tile([CHUNK, max(CG, 2 if CGMIX else 1)
                                  * G * CHUNK], bf16, tag="pt", bufs=8)
                    nc.scalar.activation(
                        out=pt[:, 0:glen * fcols]
                            .rearrange("p (k c) -> p k c", k=glen),
                        in_=slab[:, 0:glen, 0:fcols],
                        func=mybir.ActivationFunctionType.Exp)
                    pts.append((pt, c0, glen))

                if MASK_MODE == "dve":
                    # causal mask on the diagonal chunk (post-exp)
                    pt_j, c0_j, _ = pts[-1]
                    diag_off = (j - c0_j) * fcols
                    diag = pt_j[:, diag_off: diag_off + fcols] \
                        .rearrange("p (g t) -> p g t", g=G)
                    mi = mask_state["idx"]
                    gp_due = int(round((mi + 1) * MASK_GP_FRAC)) \
                        - int(round(mi * MASK_GP_FRAC))
                    eng = nc.gpsimd if gp_due else nc.vector
                    eng.tensor_mul(out=diag, in0=diag, in1=m_ap)
                    mask_state["idx"] = mi + 1
                return pts

            def stage2(i, kv_i, j, pts):
                kk = K[i]
                n = nr[(i, kv_i, j)]
                fcols = G * n
                vt = vO_t[(i, kv_i)]
                if (i, kv_i) not in ost_t:
                    ost = po.tile([CHUNK, kk * G * CHUNK], bf16,
                                  tag=f"ost_{i}_{kv_i}", bufs=2,
                                  name=f"ost_{i}_{kv_i}")
                    ost_t[(i, kv_i)] = ost
                ost = ost_t[(i, kv_i)]

                ot = psum_o.tile([CHUNK, 2, 512], f32, tag="ot")
                for c in range(j + 1):
                    pt, c0, _ = pts[c // (2 if CGMIX else CG)]
                    poff = (c - c0) * fcols
                    vsl = vt[:, c * 130:c * 130 + 130]
                    for g in range(G):
                        nc.tensor.matmul(
                            ot[0:n, g // 2,
                               (g % 2) * 132:(g % 2) * 132 + 130],
                            pt[:, poff + g * n: poff + (g + 1) * n], vsl,
                            start=(c == 0 and g % 2 == 0),
                            stop=(c == j and g % 2 == 1))

                # normalize (DVE) into bf16 staging
                obase = j * G * CHUNK
                out_ap = bass.AP(tensor=ost.tensor,
                                 offset=ost.offset + obase,
                                 ap=[ost.ap[0], [2 * D, 2], [D, 2], [1, D]])
                num_ap = bass.AP(tensor=ot.tensor, offset=ot.offset,
                                 ap=[ot.ap[0], [512, 2], [132, 2], [1, D]])
                if NORM_DIV:
                    # single tensor_tensor divide, denominator broadcast
                    # straight out of the ot ones-column
                    den_b = bass.AP(tensor=ot.tensor, offset=ot.offset + D,
                                    ap=[ot.ap[0], [512, 2], [132, 2],
                                        [0, D]])
                    nc.vector.tensor_tensor(out=out_ap, in0=num_ap,
                                            in1=den_b,
                                            op=mybir.AluOpType.divide)
                else:
                    recip = po.tile([CHUNK, G], f32, tag="recip", bufs=4)
                    den_ap = bass.AP(tensor=ot.tensor, offset=ot.offset + D,
                                     ap=[ot.ap[0], [512, 2], [132, 2]])
                    r4 = bass.AP(tensor=recip.tensor, offset=recip.offset,
                                 ap=[recip.ap[0], [2, 2], [1, 2]])
                    nc.vector.reciprocal(out=r4, in_=den_ap)
                    r_b = bass.AP(tensor=recip.tensor, offset=recip.offset,
                                  ap=[recip.ap[0], [2, 2], [1, 2], [0, D]])
                    nc.vector.tensor_mul(out=out_ap, in0=num_ap, in1=r_b)

                done = done_t.setdefault((i, kv_i), set())
                done.add(j)
                if OUT_MODE == "slab":
                    si = slab_idx[(i, kv_i, j)]
                    nc.sync.dma_start(
                        out=out_d[si],
                        in_=ost[:, j * G * CHUNK:(j + 1) * G * CHUNK])
                elif len(done) == kk:
                    out_eng = nc.gpsimd if OUT_MODE == "ikv_pool" else nc.sync
                    si0 = slab_idx[(i, kv_i, 0)]
                    out_eng.dma_start(
                        out=out_d[si0:si0 + kk].rearrange("k p c -> p k c"),
                        in_=ost[:].rearrange("p (k c) -> p k c", k=kk))

            jorder = {g: list(range(K[ikvs[g][0]])) for g in range(len(ikvs))}
            pending = []
            for t in range(len(ikvs) + maxK - 1):
                cur = []
                for g in range(len(ikvs)):
                    jj = t - g
                    i, kv_i = ikvs[g]
                    if 0 <= jj < K[i]:
                        j = jorder[g][jj]
                        cur.append((i, kv_i, j, stage1(i, kv_i, j)))
                for (i, kv_i, j, pts) in pending:
                    stage2(i, kv_i, j, pts)
                pending = cur
            for (i, kv_i, j, pts) in pending:
                stage2(i, kv_i, j, pts)

    nc.finalize()
    return nc


# --------------------------------------------------------------------------
# entry point
# --------------------------------------------------------------------------

def kernel(query, key, value, decoder_segment_ids, _trace=False, _numpy=False,
           _qdt=QDT):
    query = np.asarray(query, np.float32)
    key = np.asarray(key, np.float32)
    value = np.asarray(value, np.float32)
    ids = np.asarray(decoder_segment_ids)
    # the block-diagonal decomposition relies on segment ids being sorted
    # (contiguous segments), as setup_inputs guarantees
    assert np.all(np.diff(ids.astype(np.int64), axis=-1) >= 0)

    runs, L, K, segs, slabs, chunks, nr, qbase, qcols = _structure(ids)
    core_ins = [_prepare_core(c, query, key, value, runs, L, K, segs, slabs,
                              chunks, nr, qbase, qcols, qdt=_qdt)
                for c in range(NCORES)]

    if _numpy:
        outs = [_numpy_schedule(ci, L, K, segs, slabs, chunks, nr, qbase)
                for ci in core_ins]
        return _assemble(outs, runs, slabs, nr)

    from concourse.bass_utils import run_bass_kernel_spmd

    cache_key = (tuple(L), _qdt)
    if cache_key not in _PROGRAM_CACHE:
        _PROGRAM_CACHE[cache_key] = _build_program(
            L, K, segs, slabs, chunks, nr, qbase, qcols, qdt=_qdt)
    nc = _PROGRAM_CACHE[cache_key]

    in_maps = [_pack_core(ci, K, slabs, chunks, nr, qbase, qdt=_qdt)
               for ci in core_ins]
    res = run_bass_kernel_spmd(nc, in_maps, list(range(NCORES)), trace=_trace)
    outs = [res.results[c]["out"] for c in range(NCORES)]
    full = _assemble(outs, runs, slabs, nr)
    if _trace:
        return full, res
    return full



# revision 17
# speedup vs baseline: 1.0505x; 1.0505x over previous
"""Sharded GQA attention (causal + packed-segment mask) for 8 Trainium2 NeuronCores.

Strategy
--------
* Core c handles batch b = c//4 and KV heads {2*(c%4), 2*(c%4)+1} (8 query
  heads per core); the sequence dim stays unsharded.  decoder_segment_ids
  are sorted, so attention is block-diagonal over contiguous segments; the
  device kernel does causal-only attention per segment over 128-wide
  chunks, with the two batches' run structures unioned so all 8 cores run
  one SPMD program.
* dtypes: QK matmuls run float16 (1 col/cycle on the PE at any moving
  size, half the Q/K DMA bytes); P (post-exp) and V are bf16 so the
  130-col PV matmuls also stream 1 col/cycle; output is bf16 (host
  upcasts).  Measured end-to-end rel err 6.5e-3 (gate 2e-2).
* Q is packed host-side to only-real columns (ghost q columns of partial
  tail blocks are never computed); QK, exp, normalize and the output DMA
  are trimmed accordingly.  Zero-padded K rows self-neutralise (S=0 ->
  P=1 but V rows and the ones-column are zero), so no segment/ghost
  masking is needed anywhere.
* The causal mask inside each diagonal 128x128 block is a single shared
  additive bf16 NEG tile accumulated into the diagonal chunk's S by an
  identity-stationary matmul in the SAME PSUM accumulation group as the
  QK (no cross-engine hop; exp then emits exact zeros).
* Per-chunk S lives in its own PSUM bank (CG=1, 4-buffer pool) and exp
  runs per chunk on ScalarE; softmax denominators fall out of the PV
  matmuls via a bf16 ones-column appended to V (P^T-stationary, output
  [128, 2, 512] 2-bank psum, double-buffered); the normalize is one
  reciprocal + one 4D broadcast tensor_mul per slab on DVE.
* Emission is a software-pipelined wavefront: the 6 independent (i,kv)
  streams are staggered by one t-block step, and stage1 (QK+mask+exp) of
  step t is emitted before stage2 (PV+normalize) of step t-1, so every
  in-order engine queue always holds dependency-resolved work.
* DMA-issue overhead (~1.2us per DMA) and the serial input phase dominate
  the single-shot time, so per-(i,kv) inputs (K^T, packed Q^T, V) ride in
  one uint16-packed DMA with bitcast views (split k+q|v, extra-fine for
  the first stream), mask/ident load once from the ACT queue, outputs
  leave per-slab, and a few dep-free warmup matmuls ramp the PE clock
  during the input-DMA dead zone.

Measured on the 8 axon-tunneled trn2 NeuronCores (two-point For_i-looped
timing, RPC-drift-immune): 32669 ns per invocation vs 102159 ns baseline
(3.13x); rel err 6.488e-3.
"""

import math

import numpy as np
import ml_dtypes

B, T, NQ, NKV, D = 2, 1024, 32, 8, 128
G = NQ // NKV
NCORES = 8
KV_PER_CORE = NKV // (NCORES // B)
CHUNK = 128
BF16 = ml_dtypes.bfloat16

QDT = "f16"           # "f32r" or "f16" for the QK matmul dtype
MASK_MODE = "pe"      # "pe": additive NEG mask matmul fused into the QK
                      # accumulation; "dve": 0/1 multiply post-exp
MASK_GP_FRAC = 0.72   # dve mode: fraction of mask multiplies on GPSIMD
NEG = -1.0e9
CG = 1                # chunks per PSUM slab tile (banks each)
SLAB_BUFS = 4         # psum_s pool buffers
OT_BUFS = 2           # psum_o pool buffers
DMA_SPLIT = "first"   # input DMA pieces: "all" = (k+q | v) per (i,kv),
                      # "first" = split only stream 0, False = whole
OUT_MODE = "ikv_pool"  # "slab": per-slab DMAs on SP; "ikv": one per (i,kv)
                       # on SP; "ikv_pool": one per (i,kv) on Pool/SWDGE
                       # (bypasses the shared HWDGE + SP sequencer)
DMA_QUEUES = 1        # spread input DMAs across SP/ACT HWDGE queues
WARMUP_MM = 10        # dummy matmuls at t=0 of a single-shot build (ramp
                      # the PE clock during the input-DMA dead zone)
WARMUP_LOOP = 0       # same, inside a For_i timed body (steady state keeps
                      # the PE warm across iterations)
INPUT_BUFS = 2        # per-(i,kv) input tile buffers; 2 lets iteration n+1
                      # prefetch its inputs under iteration n's compute
LAYOUT = "interleaved"  # packed-input column order: "interleaved"
                        # ([k0|q0|k1|q1|...|v], consumption order) or
                        # "flat" ([k|q|v] blocks)
NORM_DIV = False       # normalize as one TT divide (vs reciprocal+multiply)
CGMIX = False          # mixed slab tiles: chunk PAIRS share a 2-bank tile
                      # + one exp; singles keep 1-bank tiles (24 vs 36 exps)
SLAB_TILE = None       # "j": slab j gets ONE (j+1)-bank PSUM tile and ONE
                      # exp over all its chunks (18 exps, 6 slab banks,
                      # needs OT_BUFS=1); None: CG/CGMIX per-chunk tiles
HINT_ENGINES = True    # branch-prefetch hints on the For_i back-edge so
                      # engine sequencers don't stall at the loop branch
STAGGERED_RESET = True  # For_i staggered semaphore reset instead of a hard
                        # all-engine barrier between iterations

_PROGRAM_CACHE = {}


# --------------------------------------------------------------------------
# host-side structure
# --------------------------------------------------------------------------

def _runs(seg_row):
    d = np.flatnonzero(np.diff(seg_row) != 0)
    starts = np.concatenate(([0], d + 1))
    ends = np.concatenate((d + 1, [len(seg_row)]))
    return [(int(s), int(e - s)) for s, e in zip(starts, ends)]


def _structure(ids):
    runs = [_runs(np.asarray(ids[b])) for b in range(B)]
    n_seg = max(len(r) for r in runs)
    L = [max((r[i][1] for r in runs if len(r) > i), default=0) for i in range(n_seg)]
    K = [math.ceil(l / CHUNK) for l in L]
    segs = [i for i in range(n_seg) if K[i] > 0]
    slabs = [(i, kv_i, j) for i in segs for kv_i in range(KV_PER_CORE)
             for j in range(K[i])]
    chunks = [(i, kv_i, c) for i in segs for kv_i in range(KV_PER_CORE)
              for c in range(K[i])]
    # real (non-ghost) q columns of slab (i, kv_i, j), from the union lengths
    nr = {(i, kv_i, j): min(CHUNK, L[i] - j * CHUNK)
          for (i, kv_i, j) in slabs}
    qbase = {}
    acc = 0
    for s in slabs:
        qbase[s] = acc
        acc += G * nr[s]
    return runs, L, K, segs, slabs, chunks, nr, qbase, acc


def _ikv_layout(K, slabs, chunks, nr, qbase):
    """Per-(i,kv) packed-input column layout (units: 2-byte elements).

    Columns are packed in consumption order — [k_0|q_0|k_1|q_1|...|v] —
    so the DMA pieces stream in exactly the order compute needs them.
    koff/qoff are offsets local to the (i,kv) region; voff starts the V
    block.
    """
    chunk_idx = {c: i for i, c in enumerate(chunks)}
    ikvs = sorted({(i, kv_i) for (i, kv_i, _) in slabs})
    lay = {}
    base = 0
    for (i, kv_i) in ikvs:
        kk = K[i]
        koff, qoff = [], []
        if LAYOUT == "interleaved":
            off = 0
            for j in range(kk):
                koff.append(off)
                off += CHUNK
                qoff.append(off)
                off += G * nr[(i, kv_i, j)]
        else:  # flat: [k_0..k_{kk-1} | q_0..q_{kk-1} | v]
            off = 0
            for j in range(kk):
                koff.append(j * CHUNK)
            off = kk * CHUNK
            for j in range(kk):
                qoff.append(off)
                off += G * nr[(i, kv_i, j)]
        vcols = kk * 130
        lay[(i, kv_i)] = dict(base=base, koff=koff, qoff=qoff, voff=off,
                              vcols=vcols, icols=off + vcols,
                              ci0=chunk_idx[(i, kv_i, 0)], kk=kk)
        base += off + vcols
    return ikvs, lay, base


def _prepare_core(core, q, k, v, runs, L, K, segs, slabs, chunks, nr, qbase,
                  qcols, qdt=QDT):
    b = core // (NCORES // B)
    kv_heads = [KV_PER_CORE * (core % (NCORES // B)) + x for x in range(KV_PER_CORE)]
    rb = runs[b]
    np_qdt = np.float32 if qdt == "f32r" else np.float16

    def seg_info(i):
        if i < len(rb):
            return rb[i]
        return (0, 0)

    qT = np.zeros((D, qcols), np_qdt)
    for s in slabs:
        i, kv_i, j = s
        a, lb = seg_info(i)
        t0 = j * CHUNK
        n_real = min(nr[s], max(lb - t0, 0))
        if n_real > 0:
            base = qbase[s]
            for g in range(G):
                h = G * kv_heads[kv_i] + g
                blk = q[b, a + t0:a + t0 + n_real, h, :]  # [n_real, D]
                qT[:, base + g * nr[s]: base + g * nr[s] + n_real] = blk.T

    kT = np.zeros((D, len(chunks) * CHUNK), np_qdt)
    vO = np.zeros((CHUNK, len(chunks) * 130), BF16)
    for ci, (i, kv_i, c) in enumerate(chunks):
        a, lb = seg_info(i)
        s0 = c * CHUNK
        n_real = min(CHUNK, lb - s0)
        if n_real > 0:
            kvh = kv_heads[kv_i]
            kT[:, ci * CHUNK: ci * CHUNK + n_real] = \
                k[b, a + s0:a + s0 + n_real, kvh, :].T.astype(np_qdt)
            vO[:n_real, ci * 130: ci * 130 + D] = \
                v[b, a + s0:a + s0 + n_real, kvh, :].astype(BF16)
            vO[:n_real, ci * 130 + D] = BF16(1.0)

    sr = np.arange(CHUNK)
    if MASK_MODE == "pe":
        keep = np.where(sr[:, None] > sr[None, :], np.float32(NEG),
                        np.float32(0.0))  # additive: NEG where t < s
    else:
        keep = (sr[:, None] <= sr[None, :]).astype(np.float32)  # 0/1 keep
    mask = np.concatenate([keep] * G, axis=1).astype(BF16)  # [s, g*128 + t]

    return {"qT": qT, "kT": kT, "vO": vO, "mask": mask,
            "ident": np.eye(CHUNK, dtype=BF16)}


def _pack_core(ci, K, slabs, chunks, nr, qbase, qdt=QDT):
    """Build the device in_map from the logical per-core arrays."""
    ikvs, lay, total = _ikv_layout(K, slabs, chunks, nr, qbase)
    if qdt == "f16":
        inb = np.zeros((CHUNK, total), np.uint16)
        for ikv in ikvs:
            l = lay[ikv]
            b0 = l["base"]
            ci0, kk = l["ci0"], l["kk"]
            for j in range(kk):
                inb[:, b0 + l["koff"][j]: b0 + l["koff"][j] + CHUNK] = \
                    ci["kT"][:, (ci0 + j) * CHUNK:(ci0 + j + 1) * CHUNK] \
                    .view(np.uint16)
                s = (ikv[0], ikv[1], j)
                qn = G * nr[s]
                inb[:, b0 + l["qoff"][j]: b0 + l["qoff"][j] + qn] = \
                    ci["qT"][:, qbase[s]: qbase[s] + qn].view(np.uint16)
            inb[:, b0 + l["voff"]: b0 + l["voff"] + l["vcols"]] = \
                ci["vO"][:, ci0 * 130:(ci0 + kk) * 130].view(np.uint16)
        mi = np.concatenate([ci["mask"], ci["ident"]], axis=1)
        return {"inb": inb, "mi": mi}
    mi = np.concatenate([ci["mask"], ci["ident"]], axis=1)
    return {"kT": ci["kT"], "qT": ci["qT"], "vO": ci["vO"], "mi": mi}


def _assemble(outs, runs, slabs, nr):
    full = np.zeros((B, T, NQ, D), np.float32)
    for core in range(NCORES):
        b = core // (NCORES // B)
        kv_heads = [KV_PER_CORE * (core % (NCORES // B)) + x
                    for x in range(KV_PER_CORE)]
        res = outs[core]  # [NSLAB, 128, 512] bf16
        rb = runs[b]
        for si, (i, kv_i, j) in enumerate(slabs):
            if i >= len(rb):
                continue
            a, lb = rb[i]
            t0 = j * CHUNK
            n_real = min(CHUNK, lb - t0)
            if n_real <= 0:
                continue
            for g in range(G):
                h = G * kv_heads[kv_i] + g
                full[b, a + t0:a + t0 + n_real, h, :] = \
                    res[si, :n_real, g * CHUNK:g * CHUNK + D].astype(np.float32)
    return full


# --------------------------------------------------------------------------
# numpy emulation of the device schedule (debug/validation only)
# --------------------------------------------------------------------------

def _numpy_schedule(ins, L, K, segs, slabs, chunks, nr, qbase):
    chunk_idx = {c: i for i, c in enumerate(chunks)}
    qT = ins["qT"].astype(np.float32)
    kT = ins["kT"].astype(np.float32)
    vO = ins["vO"].astype(np.float32)
    mask = ins["mask"].astype(np.float32)
    out = np.zeros((len(slabs), CHUNK, G * CHUNK), BF16)
    for si, (i, kv_i, j) in enumerate(slabs):
        n = nr[(i, kv_i, j)]
        qt = qT[:, qbase[(i, kv_i, j)]: qbase[(i, kv_i, j)] + G * n]  # [d, 4n]
        ot = np.zeros((CHUNK, G, 130), np.float32)
        for c in range(j + 1):
            ci = chunk_idx[(i, kv_i, c)]
            lhsT = kT[:, ci * CHUNK:(ci + 1) * CHUNK]          # [d, s]
            S = lhsT.T @ qt                                    # [s, 4n]
            m = np.concatenate([mask[:, :n]] * G, axis=1)      # [s, 4n]
            if MASK_MODE == "pe":
                if c == j:
                    S = S + m
                P = np.exp(S)
            else:
                P = np.exp(S)
                if c == j:
                    P = P * m
            P = P.astype(BF16).astype(np.float32)
            vo = vO[:, ci * 130:ci * 130 + 130]                # [s, 130]
            for g in range(G):
                ot[:n, g, :] += P[:, g * n:(g + 1) * n].T @ vo
        den = ot[:, :, D]
        with np.errstate(divide="ignore", invalid="ignore"):
            recip = 1.0 / den
            norm = ot[:, :, :D] * recip[:, :, None]
        out[si, :, :] = norm.reshape(CHUNK, G * D).astype(BF16)
    return out


# --------------------------------------------------------------------------
# bass program
# --------------------------------------------------------------------------

def _build_program(L, K, segs, slabs, chunks, nr, qbase, qcols, qdt=QDT,
                   loop_n=0, unroll=1):
    import contextlib

    import concourse.bacc as bacc
    import concourse.bass as bass
    import concourse.tile as tile
    from concourse import mybir

    f32 = mybir.dt.float32
    bf16 = mybir.dt.bfloat16
    u16 = mybir.dt.uint16
    f16pack = qdt == "f16"
    mm_dt = mybir.dt.float32r if qdt == "f32r" else mybir.dt.float16
    maxK = max(K[i] for i in segs)
    nslab = len(slabs)
    nchunk = len(chunks)
    ikvs, lay, packed_cols = _ikv_layout(K, slabs, chunks, nr, qbase)

    nc = bacc.Bacc()
    if f16pack:
        inb_d = nc.dram_tensor("inb", [CHUNK, packed_cols], u16,
                               kind="ExternalInput")
    else:
        qT_d = nc.dram_tensor("qT", [D, qcols], mm_dt, kind="ExternalInput")
        kT_d = nc.dram_tensor("kT", [D, nchunk * CHUNK], mm_dt,
                              kind="ExternalInput")
        vO_d = nc.dram_tensor("vO", [CHUNK, nchunk * 130], bf16,
                              kind="ExternalInput")
    mi_d = nc.dram_tensor("mi", [CHUNK, G * CHUNK + CHUNK], bf16,
                          kind="ExternalInput")
    out_d = nc.dram_tensor("out", [nslab, CHUNK, G * CHUNK], bf16,
                           kind="ExternalOutput")
    slab_idx = {s: i for i, s in enumerate(slabs)}

    with tile.TileContext(nc) as tc:
      with tc.tile_pool(name="pin", bufs=1) as pin, \
           tc.tile_pool(name="pp", bufs=3) as pp, \
           tc.tile_pool(name="po", bufs=2) as po, \
           tc.tile_pool(name="psum_s", bufs=SLAB_BUFS, space="PSUM") as psum_s, \
           tc.tile_pool(name="psum_o", bufs=OT_BUFS, space="PSUM") as psum_o:
        # loop-invariant: causal mask + identity in ONE tile/DMA; issued from
        # the ACT queue so the SP queue's first input DMA is not delayed
        mi_t = pin.tile([CHUNK, G * CHUNK + CHUNK], bf16, tag="mi")
        nc.scalar.dma_start(out=mi_t[:], in_=mi_d[:])
        mask_t = mi_t[:, 0:G * CHUNK]
        ident_t = mi_t[:, G * CHUNK:G * CHUNK + CHUNK]
        warm_t = pin.tile([CHUNK, CHUNK], bf16, tag="warm")
        nc.vector.memset(warm_t[:], 0.0)
        warm_n = WARMUP_LOOP if loop_n else WARMUP_MM
        hints = tuple(mybir.ALL_ENGINES) if HINT_ENGINES else ()
        with (tc.For_i(0, loop_n, 1, staggered_reset=STAGGERED_RESET,
                       hint_engines=hints)
              if loop_n else contextlib.nullcontext()):
          for _it in range(max(1, unroll)):
            if warm_n:
                # dep-free dummy matmuls (uninitialized operands, result
                # overwritten): keep the PE busy during the input-DMA head
                # so the HAM/pstate clock is warm for the first real QK
                if SLAB_TILE == "j":
                    wslab = psum_s.tile([CHUNK, maxK, G * CHUNK], f32,
                                        tag=f"slabJ{maxK - 1}", bufs=1,
                                        name=f"slabJ{maxK - 1}")
                else:
                    wslab = psum_s.tile(
                        [CHUNK, 1 if CGMIX else CG, G * CHUNK], f32,
                        tag="slab", bufs=2 if CGMIX else SLAB_BUFS,
                        name="wslab")
                for w in range(warm_n):
                    nc.tensor.matmul(wslab[:, 0, 0:CHUNK], warm_t[:],
                                     warm_t[:], start=True, stop=True)
            # one packed input DMA per (i,kv), in consumption order, so the
            # For_i loop's n+1 DMAs overlap iteration n's compute
            kT_t, qT_t, vO_t = {}, {}, {}
            for gi_, ikv in enumerate(ikvs):
                dma_eng = (nc.gpsimd if (DMA_QUEUES > 1 and gi_ % 2 == 1)
                           else nc.sync)
                l = lay[ikv]
                kk = l["kk"]
                if f16pack:
                    icols = l["icols"]
                    voff = l["voff"]
                    it = pin.tile([CHUNK, icols], u16,
                                  tag=f"in_{ikv[0]}_{ikv[1]}",
                                  bufs=INPUT_BUFS)
                    if DMA_SPLIT == "first" and ikv == ikvs[0]:
                        # first stream lands [k0|q0] first so the first
                        # QK starts ASAP; rest follows in one piece
                        p1 = l["qoff"][0] + G * nr[(ikv[0], ikv[1], 0)]
                        dma_eng.dma_start(
                            out=it[:, 0:p1],
                            in_=inb_d[:, l["base"]: l["base"] + p1])
                        dma_eng.dma_start(
                            out=it[:, p1:icols],
                            in_=inb_d[:, l["base"] + p1: l["base"] + icols])
                    elif DMA_SPLIT == "all":
                        dma_eng.dma_start(
                            out=it[:, 0:voff],
                            in_=inb_d[:, l["base"]: l["base"] + voff])
                        dma_eng.dma_start(
                            out=it[:, voff:icols],
                            in_=inb_d[:, l["base"] + voff: l["base"] + icols])
                    else:
                        dma_eng.dma_start(
                            out=it[:],
                            in_=inb_d[:, l["base"]: l["base"] + icols])
                    kT_t[ikv] = [it[:, l["koff"][j]: l["koff"][j] + CHUNK]
                                 .bitcast(mm_dt) for j in range(kk)]
                    qT_t[ikv] = [
                        it[:, l["qoff"][j]:
                           l["qoff"][j] + G * nr[(ikv[0], ikv[1], j)]]
                        .bitcast(mm_dt) for j in range(kk)]
                    vO_t[ikv] = it[:, voff: icols].bitcast(bf16)
                else:
                    ci0 = l["ci0"]
                    s0 = (ikv[0], ikv[1], 0)
                    qlen = sum(G * nr[(ikv[0], ikv[1], j)] for j in range(kk))
                    kt = pin.tile([D, kk * CHUNK], mm_dt,
                                  tag=f"kT_{ikv[0]}_{ikv[1]}")
                    nc.sync.dma_start(
                        out=kt[:], in_=kT_d[:, ci0 * CHUNK:(ci0 + kk) * CHUNK])
                    kT_t[ikv] = [kt[:, j * CHUNK:(j + 1) * CHUNK]
                                 for j in range(kk)]
                    qt = pin.tile([D, qlen], mm_dt,
                                  tag=f"qT_{ikv[0]}_{ikv[1]}")
                    nc.sync.dma_start(
                        out=qt[:], in_=qT_d[:, qbase[s0]: qbase[s0] + qlen])
                    qT_t[ikv] = [
                        qt[:, qbase[(ikv[0], ikv[1], j)] - qbase[s0]:
                           qbase[(ikv[0], ikv[1], j)] - qbase[s0]
                           + G * nr[(ikv[0], ikv[1], j)]]
                        for j in range(kk)]
                    vt = pin.tile([CHUNK, kk * 130], bf16,
                                  tag=f"vO_{ikv[0]}_{ikv[1]}")
                    nc.sync.dma_start(
                        out=vt[:], in_=vO_d[:, ci0 * 130:(ci0 + kk) * 130])
                    vO_t[ikv] = vt[:]

            # ---- software-pipelined wavefront over the (i,kv) streams ----
            # Streams are independent; stagger them by one j-step and emit
            # stage1 (QK+exp+mask) of step t before stage2 (PV+normalize)
            # of step t-1, so every engine's in-order queue always holds
            # dependency-resolved work.
            mask_state = {"idx": 0}
            ost_t = {}
            done_t = {}

            def stage1(i, kv_i, j):
                kt = kT_t[(i, kv_i)]
                n = nr[(i, kv_i, j)]
                fcols = G * n
                qt = qT_t[(i, kv_i)][j]
                m_ap = bass.AP(tensor=mask_t.tensor, offset=mask_t.offset,
                               ap=[mask_t.ap[0], [0, G], [1, n]])
                pts = []  # (pt_tile, c0, glen)
                step = (j + 1) if SLAB_TILE == "j" else (2 if CGMIX else CG)
                for c0 in range(0, j + 1, step):
                    glen = min(step, j + 1 - c0)
                    if SLAB_TILE == "j":
                        slab = psum_s.tile([CHUNK, j + 1, G * CHUNK], f32,
                                           tag=f"slabJ{j}", bufs=1,
                                           name=f"slabJ{j}")
                    elif CGMIX and glen == 2:
                        # paired chunks share a 2-bank tile and ONE exp
                        slab = psum_s.tile([CHUNK, 2, G * CHUNK], f32,
                                           tag="slab2", bufs=1, name="slab2")
                    elif CGMIX:
                        slab = psum_s.tile([CHUNK, 1, G * CHUNK], f32,
                                           tag="slab", bufs=2, name="slab")
                    else:
                        slab = psum_s.tile([CHUNK, CG, G * CHUNK], f32,
                                           tag="slab", name="slab")
                    for gi in range(glen):
                        c = c0 + gi
                        masked = MASK_MODE == "pe" and c == j
                        nc.tensor.matmul(
                            slab[:, gi, 0:fcols],
                            kt[c], qt,
                            start=True, stop=not masked)
                        if masked:
                            # accumulate the additive NEG causal mask into
                            # the diagonal chunk's S (same PSUM group, no
                            # cross-engine hop; exp then emits exact zeros)
                            sl3 = slab[:, gi, 0:fcols] \
                                .rearrange("p (g t) -> p g t", g=G)
                            nc.tensor.matmul(sl3, ident_t[:], m_ap,
                                             start=False, stop=True)
                    ptk = maxK if SLAB_TILE == "j" else max(
                        CG, 2 if CGMIX else 1)
                    pt = pp.tile([CHUNK, ptk * G * CHUNK], bf16, tag="pt",
                                 bufs=8 if ptk == 1 else 6)
                    nc.scalar.activation(
                        out=pt[:, 0:glen * fcols]
                            .rearrange("p (k c) -> p k c", k=glen),
                        in_=slab[:, 0:glen, 0:fcols],
                        func=mybir.ActivationFunctionType.Exp)
                    pts.append((pt, c0, glen))

                if MASK_MODE == "dve":
                    # causal mask on the diagonal chunk (post-exp)
                    pt_j, c0_j, _ = pts[-1]
                    diag_off = (j - c0_j) * fcols
                    diag = pt_j[:, diag_off: diag_off + fcols] \
                        .rearrange("p (g t) -> p g t", g=G)
                    mi = mask_state["idx"]
                    gp_due = int(round((mi + 1) * MASK_GP_FRAC)) \
                        - int(round(mi * MASK_GP_FRAC))
                    eng = nc.gpsimd if gp_due else nc.vector
                    eng.tensor_mul(out=diag, in0=diag, in1=m_ap)
                    mask_state["idx"] = mi + 1
                return pts

            def stage2(i, kv_i, j, pts):
                kk = K[i]
                n = nr[(i, kv_i, j)]
                fcols = G * n
                vt = vO_t[(i, kv_i)]
                if (i, kv_i) not in ost_t:
                    ost = po.tile([CHUNK, kk * G * CHUNK], bf16,
                                  tag=f"ost_{i}_{kv_i}", bufs=2,
                                  name=f"ost_{i}_{kv_i}")
                    ost_t[(i, kv_i)] = ost
                ost = ost_t[(i, kv_i)]

                ot = psum_o.tile([CHUNK, 2, 512], f32, tag="ot")
                pstep = (j + 1) if SLAB_TILE == "j" else (2 if CGMIX else CG)
                for c in range(j + 1):
                    pt, c0, _ = pts[c // pstep]
                    poff = (c - c0) * fcols
                    vsl = vt[:, c * 130:c * 130 + 130]
                    for g in range(G):
                        nc.tensor.matmul(
                            ot[0:n, g // 2,
                               (g % 2) * 132:(g % 2) * 132 + 130],
                            pt[:, poff + g * n: poff + (g + 1) * n], vsl,
                            start=(c == 0 and g % 2 == 0),
                            stop=(c == j and g % 2 == 1))

                # normalize (DVE) into bf16 staging
                obase = j * G * CHUNK
                out_ap = bass.AP(tensor=ost.tensor,
                                 offset=ost.offset + obase,
                                 ap=[ost.ap[0], [2 * D, 2], [D, 2], [1, D]])
                num_ap = bass.AP(tensor=ot.tensor, offset=ot.offset,
                                 ap=[ot.ap[0], [512, 2], [132, 2], [1, D]])
                if NORM_DIV:
                    # single tensor_tensor divide, denominator broadcast
                    # straight out of the ot ones-column
                    den_b = bass.AP(tensor=ot.tensor, offset=ot.offset + D,
                                    ap=[ot.ap[0], [512, 2], [132, 2],
                                        [0, D]])
                    nc.vector.tensor_tensor(out=out_ap, in0=num_ap,
                                            in1=den_b,
                                            op=mybir.AluOpType.divide)
                else:
                    recip = po.tile([CHUNK, G], f32, tag="recip", bufs=4)
                    den_ap = bass.AP(tensor=ot.tensor, offset=ot.offset + D,
                                     ap=[ot.ap[0], [512, 2], [132, 2]])
                    r4 = bass.AP(tensor=recip.tensor, offset=recip.offset,
                                 ap=[recip.ap[0], [2, 2], [1, 2]])
                    nc.vector.reciprocal(out=r4, in_=den_ap)
                    r_b = bass.AP(tensor=recip.tensor, offset=recip.offset,
                                  ap=[recip.ap[0], [2, 2], [1, 2], [0, D]])
                    nc.vector.tensor_mul(out=out_ap, in0=num_ap, in1=r_b)

                done = done_t.setdefault((i, kv_i), set())
                done.add(j)
                if OUT_MODE == "slab":
                    si = slab_idx[(i, kv_i, j)]
                    nc.sync.dma_start(
                        out=out_d[si],
                        in_=ost[:, j * G * CHUNK:(j + 1) * G * CHUNK])
                elif len(done) == kk:
                    out_eng = nc.gpsimd if OUT_MODE == "ikv_pool" else nc.sync
                    si0 = slab_idx[(i, kv_i, 0)]
                    out_eng.dma_start(
                        out=out_d[si0:si0 + kk].rearrange("k p c -> p k c"),
                        in_=ost[:].rearrange("p (k c) -> p k c", k=kk))

            jorder = {g: list(range(K[ikvs[g][0]])) for g in range(len(ikvs))}
            pending = []
            for t in range(len(ikvs) + maxK - 1):
                cur = []
                for g in range(len(ikvs)):
                    jj = t - g
                    i, kv_i = ikvs[g]
                    if 0 <= jj < K[i]:
                        j = jorder[g][jj]
                        cur.append((i, kv_i, j, stage1(i, kv_i, j)))
                for (i, kv_i, j, pts) in pending:
                    stage2(i, kv_i, j, pts)
                pending = cur
            for (i, kv_i, j, pts) in pending:
                stage2(i, kv_i, j, pts)

    nc.finalize()
    return nc


# --------------------------------------------------------------------------
# entry point
# --------------------------------------------------------------------------

def kernel(query, key, value, decoder_segment_ids, _trace=False, _numpy=False,
           _qdt=QDT):
    query = np.asarray(query, np.float32)
    key = np.asarray(key, np.float32)
    value = np.asarray(value, np.float32)
    ids = np.asarray(decoder_segment_ids)
    # the block-diagonal decomposition relies on segment ids being sorted
    # (contiguous segments), as setup_inputs guarantees
    assert np.all(np.diff(ids.astype(np.int64), axis=-1) >= 0)

    runs, L, K, segs, slabs, chunks, nr, qbase, qcols = _structure(ids)
    core_ins = [_prepare_core(c, query, key, value, runs, L, K, segs, slabs,
                              chunks, nr, qbase, qcols, qdt=_qdt)
                for c in range(NCORES)]

    if _numpy:
        outs = [_numpy_schedule(ci, L, K, segs, slabs, chunks, nr, qbase)
                for ci in core_ins]
        return _assemble(outs, runs, slabs, nr)

    from concourse.bass_utils import run_bass_kernel_spmd

    cache_key = (tuple(L), _qdt)
    if cache_key not in _PROGRAM_CACHE:
        _PROGRAM_CACHE[cache_key] = _build_program(
            L, K, segs, slabs, chunks, nr, qbase, qcols, qdt=_qdt)
    nc = _PROGRAM_CACHE[cache_key]

    in_maps = [_pack_core(ci, K, slabs, chunks, nr, qbase, qdt=_qdt)
               for ci in core_ins]
    res = run_bass_kernel_spmd(nc, in_maps, list(range(NCORES)), trace=_trace)
    outs = [res.results[c]["out"] for c in range(NCORES)]
    full = _assemble(outs, runs, slabs, nr)
    if _trace:
        return full, res
    return full

